# revision 11
# baseline (speedup 1.0000x reference)
"""Multi-head attention (B=4, S=1024, D=1024, H=16) on 8 TRN2 NeuronCores.

Sharding: batch (4) x head-half (2) -> 8 cores, zero cross-core traffic.
Core c handles batch b = c // 2 and heads [hh*8, hh*8+8) where hh = c % 2.
Each core computes a partial output y_part[s, e] (its 512 channels fed
through its slice of Wo); the host sums the two partials per batch and
adds the bias terms.

On-device pipeline per core (matmul operands bf16, accumulation fp32):
  QT_j = Wq_j' @ xq    [128 dout, 1024 s]   (weights pre-scaled 1/sqrt(dk))
  KT_j = Wk_j' @ xk    [128 dout, 1024 s]
  V'   = xv @ Wv'      [s, 512] natural layout + per-head ones column
  per (pair j, q-chunk qn of 512), per k-tile (8x128), heads row-packed:
    ST   = KhT.T @ QhT            [k 128, q 512]   (K=64, tile_position;
                                   the two heads' streams run concurrently)
    E    = exp(ST + mask_bias)    (ACT, fused mask)
    psO += Vaug.T @ E             [65, q 512]  rows 0-63 = out_h^T, row 64 = denom
    concatT = psO[0:64] * recip(psO[64])   (DVE reciprocal_approx_fast +
                                   gpsimd broadcast + DVE multiply)
  y = concatT.T @ Wo'  [1024 s, 1024 e]

Scheduling: the ACT exp stream (64 x ~1.1us) paces the attention inner
loop, so the PE issue stream weaves "filler" matmul quanta (V projection,
next pair's Q/K projections, first half of the output projection) between
the score matmuls to keep the PE busy during pssp/exp waits. The Scalar
queue carries no DMAs after startup so it runs exps back-to-back.
"""

import os
import sys

sys.path.insert(0, "/opt/trn_rl_repo")

import numpy as np
import ml_dtypes

BF16 = ml_dtypes.bfloat16

B, S, D = 4, 1024, 1024
HEADS = 16
DK = 64
P = 128
NCORES = 8
DCH = D // P       # 8 contraction chunks
PAIRS = 4          # head-pairs per core (8 heads / 2)
QN = 2             # q 512-chunks
KT = 8             # k tiles of 128
VW = 65            # V channels per head + ones column

_STATE = {}


def _build():
    """Build + compile the per-core Bass program (cached)."""
    if "nc" in _STATE:
        return _STATE["nc"]

    import concourse.bass as bass  # noqa: F401
    import concourse.mybir as mybir
    from concourse import bacc
    from concourse import tile

    f32 = mybir.dt.float32
    bf16 = mybir.dt.bfloat16
    AF = mybir.ActivationFunctionType
    ALU = mybir.AluOpType

    # Pin Exp/Ln to the one activation table containing both, so the
    # table-load pass never alternates tables between the softmax exp and
    # the ln/exp reciprocal (each ACT_TABLE_LOAD costs ~1.3us).
    _orig_tables = bacc.get_activation_tables

    def _pinned_tables(arch):
        t = dict(_orig_tables(arch))
        target = "natural_log_exp_and_others"
        if target in t:
            for k in t:
                if k != target:
                    t[k] = t[k] - {AF.Exp, AF.Ln}
        return t

    bacc.get_activation_tables = _pinned_tables

    nc = bacc.Bacc("TRN2", target_bir_lowering=False, debug=False)

    xq_d = nc.dram_tensor("xq", [D, S], bf16, kind="ExternalInput")
    xk_d = nc.dram_tensor("xk", [D, S], bf16, kind="ExternalInput")
    xv_d = nc.dram_tensor("xv", [D, S], bf16, kind="ExternalInput")
    wq_d = nc.dram_tensor("wq", [PAIRS, D, P], bf16, kind="ExternalInput")
    wk_d = nc.dram_tensor("wk", [PAIRS, D, P], bf16, kind="ExternalInput")
    wv_d = nc.dram_tensor("wv", [D, 512], bf16, kind="ExternalInput")
    wo_d = nc.dram_tensor("wo", [512, D], bf16, kind="ExternalInput")
    bq_d = nc.dram_tensor("bq", [P, PAIRS], f32, kind="ExternalInput")
    bk_d = nc.dram_tensor("bk", [P, PAIRS], f32, kind="ExternalInput")
    mb_d = nc.dram_tensor("mb", [P, KT], f32, kind="ExternalInput")
    y_d = nc.dram_tensor("y", [S, D], f32, kind="ExternalOutput")

    from contextlib import ExitStack

    with tile.TileContext(nc) as tc, ExitStack() as ctx:
        const = ctx.enter_context(tc.tile_pool(name="const", bufs=1))
        # Resident tensors
        wv_sb = const.tile([P, DCH, 512], bf16)
        wo_sb = const.tile([P, PAIRS, D], bf16)
        xq_sb = const.tile([P, DCH, S], bf16)
        xk_sb = const.tile([P, DCH, S], bf16)
        xv_sb = const.tile([P, DCH, S], bf16)
        v_sb = const.tile([P, KT, 8 * VW], bf16)
        cat_sb = const.tile([P, PAIRS, S], bf16)
        bq_sb = const.tile([P, PAIRS], f32)
        bk_sb = const.tile([P, PAIRS], f32)
        mb_sb = const.tile([P, KT], f32)

        # Pools
        wqp = ctx.enter_context(tc.tile_pool(name="wqp", bufs=2))
        wkp = ctx.enter_context(tc.tile_pool(name="wkp", bufs=2))
        qtp = ctx.enter_context(tc.tile_pool(name="qtp", bufs=2))
        ktp = ctx.enter_context(tc.tile_pool(name="ktp", bufs=2))
        epool = ctx.enter_context(tc.tile_pool(name="epool", bufs=16))
        rpool = ctx.enter_context(tc.tile_pool(name="rpool", bufs=4))
        r2pool = ctx.enter_context(tc.tile_pool(name="r2pool", bufs=4))
        ypool = ctx.enter_context(tc.tile_pool(name="ypool", bufs=3))
        psacc = ctx.enter_context(tc.tile_pool(name="psacc", bufs=2, space="PSUM"))
        pssp = ctx.enter_context(tc.tile_pool(name="pssp", bufs=2, space="PSUM"))
        psop = ctx.enter_context(tc.tile_pool(name="psop", bufs=2, space="PSUM"))

        # --- Startup DMAs: one big 3D transfer per tensor, spread across
        # queues. Scalar only carries xq (it idles until the first exp).
        xq_r = xq_d.ap().rearrange("(d p) s -> p d s", p=P)
        xk_r = xk_d.ap().rearrange("(d p) s -> p d s", p=P)
        xv_r = xv_d.ap().rearrange("(d p) s -> p d s", p=P)
        wq_r = wq_d.ap().rearrange("j (d p) m -> j p d m", p=P)
        wk_r = wk_d.ap().rearrange("j (d p) m -> j p d m", p=P)

        wq_tiles = [None] * PAIRS
        wk_tiles = [None] * PAIRS

        def load_wqk(j, engine):
            wq_tiles[j] = wqp.tile([P, DCH, P], bf16, tag="wq", name=f"wq{j}")
            engine.dma_start(wq_tiles[j][:], wq_r[j])
            wk_tiles[j] = wkp.tile([P, DCH, P], bf16, tag="wk", name=f"wk{j}")
            engine.dma_start(wk_tiles[j][:], wk_r[j])

        nc.sync.dma_start(bq_sb[:], bq_d.ap())
        nc.sync.dma_start(bk_sb[:], bk_d.ap())
        nc.sync.dma_start(mb_sb[:], mb_d.ap())
        load_wqk(0, nc.sync)
        nc.scalar.dma_start(xq_sb[:], xq_r[:])
        nc.sync.dma_start(xk_sb[:], xk_r[:])
        nc.gpsimd.dma_start(xv_sb[:], xv_r[:])
        nc.gpsimd.dma_start(wv_sb[:], wv_d.ap().rearrange("(d p) m -> p d m", p=P))
        nc.gpsimd.dma_start(wo_sb[:], wo_d.ap().rearrange("(c p) e -> p c e", p=P))

        # Ones columns of V' (denominator trick). memset can't emit a
        # bf16-typed strided set here; stage ones in f32.
        ones_f32 = const.tile([P, KT, 8], f32)
        nc.vector.memset(ones_f32[:], 1.0)
        ones_view = v_sb.rearrange("p t (h c) -> p t h c", c=VW)[:, :, :, 64:65]
        nc.vector.tensor_copy(ones_view, ones_f32[:].unsqueeze(3))

        qt_tiles = [None] * PAIRS
        kt_tiles = [None] * PAIRS

        # ---- Filler machinery: a FIFO of (tag, closure), each emitting
        # ~1-2 matmuls (plus companion DVE/DMA work). feed(n) pops up to n.
        fill = []

        def feed(n):
            for _ in range(min(n, len(fill))):
                fill.pop(0)[1]()

        def drain(tag):
            """Emit every queued quantum up to and including the given tag
            (FIFO order), so later direct emissions see their writers."""
            while any(t == tag for t, _ in fill):
                fill.pop(0)[1]()

        def proj_quanta(j):
            """Q/K projection for pair j as 8 filler quanta."""
            qt = qtp.tile([P, S], bf16, tag="qt", name=f"qt{j}")
            ktt = ktp.tile([P, S], bf16, tag="kt", name=f"kt{j}")
            qt_tiles[j] = qt
            kt_tiles[j] = ktt
            out = []
            for n in range(QN):
                for which in ("q", "k"):
                    wt = wq_tiles[j] if which == "q" else wk_tiles[j]
                    xs = xq_sb if which == "q" else xk_sb
                    dst = qt if which == "q" else ktt
                    bias = bq_sb if which == "q" else bk_sb
                    ps_box = [None]

                    def first(wt=wt, xs=xs, n=n, ps_box=ps_box, j=j, which=which):
                        ps = psacc.tile([P, 512], f32, tag="acc", name=f"ps{which}{j}_{n}")
                        ps_box[0] = ps
                        for d in range(4):
                            nc.tensor.matmul(
                                ps[:], wt[:, d], xs[:, d, n * 512 : (n + 1) * 512],
                                start=(d == 0), stop=False,
                            )

                    def second(wt=wt, xs=xs, n=n, ps_box=ps_box, dst=dst, bias=bias, j=j):
                        ps = ps_box[0]
                        for d in range(4, DCH):
                            nc.tensor.matmul(
                                ps[:], wt[:, d], xs[:, d, n * 512 : (n + 1) * 512],
                                start=False, stop=(d == DCH - 1),
                            )
                        nc.vector.tensor_scalar_add(
                            dst[:, n * 512 : (n + 1) * 512], ps[:], bias[:, j : j + 1]
                        )

                    out.append((f"proj{j}", first))
                    out.append((f"proj{j}", second))
            return out

        def v_quanta():
            """V' projection as 16 filler quanta (2 per s-tile)."""
            out = []
            for st in range(KT):
                ps_box = [None]

                def first(st=st, ps_box=ps_box):
                    ps = psacc.tile([P, 512], f32, tag="acc", name=f"psv{st}")
                    ps_box[0] = ps
                    for d in range(4):
                        nc.tensor.matmul(
                            ps[:], xv_sb[:, d, st * P : (st + 1) * P], wv_sb[:, d],
                            start=(d == 0), stop=False,
                        )

                def second(st=st, ps_box=ps_box):
                    ps = ps_box[0]
                    for d in range(4, DCH):
                        nc.tensor.matmul(
                            ps[:], xv_sb[:, d, st * P : (st + 1) * P], wv_sb[:, d],
                            start=False, stop=(d == DCH - 1),
                        )
                    vview = v_sb[:, st].rearrange("p (h c) -> p h c", c=VW)
                    nc.vector.tensor_copy(
                        vview[:, :, 0:64], ps[:].rearrange("p (h c) -> p h c", c=64)
                    )

                out.append(("v", first))
                out.append(("v", second))
            return out

        y_r = y_d.ap().rearrange("(st p) e -> st p e", p=P)

        def o_quanta(st_list):
            """Output projection for the given s-tiles (1 quantum per (st, en))."""
            out = []
            for st in st_list:
                for en in range(2):
                    def q(st=st, en=en):
                        psy = psacc.tile([P, 512], f32, tag="acc", name=f"psy{st}_{en}")
                        for cc in range(PAIRS):
                            nc.tensor.matmul(
                                psy[:],
                                cat_sb[:, cc, st * P : (st + 1) * P],
                                wo_sb[:, cc, en * 512 : (en + 1) * 512],
                                start=(cc == 0), stop=(cc == PAIRS - 1),
                            )
                        ysb = ypool.tile([P, 512], f32, tag="y", name=f"y{st}_{en}")
                        nc.vector.tensor_copy(ysb[:], psy[:])
                        nc.sync.dma_start(y_r[st][:, en * 512 : (en + 1) * 512], ysb[:])

                    out.append(("o", q))
            return out

        # --- Pair-0 Q/K projections run directly (nothing to overlap yet).
        for _, q in proj_quanta(0):
            q()

        # Filler for pair 0/1 windows: V projection, then pair-1 projections.
        fill += v_quanta()
        load_wqk(1, nc.sync)
        fill += proj_quanta(1)

        # --- Head-pair attention loop with weaving ---
        for j in range(PAIRS):
            if j + 2 < PAIRS:
                load_wqk(j + 2, nc.sync)
            drain(f"proj{j}")  # ensure this pair's qt/kt writers are emitted
            qt = qt_tiles[j]
            ktt = kt_tiles[j]
            for qn in range(QN):
                if j == PAIRS - 1 and qn == 1:
                    # First s-half of the output projection only needs qn0
                    # cats (all pairs) -> weave it into the last window.
                    fill.extend(o_quanta([0, 1, 2, 3]))
                ets = []
                for kt in range(KT):
                    pss = pssp.tile([P, 2, 512], f32, tag="s", name=f"pss{j}_{qn}_{kt}")
                    for sub in range(2):
                        lo, hi = sub * 64, (sub + 1) * 64
                        nc.tensor.matmul(
                            pss[:, sub],
                            ktt[lo:hi, kt * P : (kt + 1) * P],
                            qt[lo:hi, qn * 512 : (qn + 1) * 512],
                            start=True, stop=True,
                        )
                    et = epool.tile([P, 2, 512], bf16, tag="e", name=f"e{j}_{qn}_{kt}")
                    nc.scalar.activation(
                        et[:], pss[:], AF.Exp, bias=mb_sb[:, kt : kt + 1], scale=1.0
                    )
                    ets.append(et)
                    feed(3)
                for sub in range(2):
                    h = j * 2 + sub
                    feed(4)
                    pso = psop.tile([VW, 512], f32, tag="o", name=f"pso{j}_{sub}_{qn}")
                    for kt in range(KT):
                        nc.tensor.matmul(
                            pso[:],
                            v_sb[:, kt, h * VW : (h + 1) * VW],
                            ets[kt][:, sub],
                            start=(kt == 0), stop=(kt == KT - 1),
                        )
                    lo, hi = sub * 64, (sub + 1) * 64
                    # Normalize: 1/denom as exp(-ln(denom)) on ACT, gpsimd
                    # partition-broadcast, then the multiply evicts psO.
                    lrow = rpool.tile([1, 512], f32, tag="l", name=f"l{j}_{sub}_{qn}")
                    nc.scalar.activation(lrow[:], pso[64:65, :], AF.Ln)
                    rrow = rpool.tile([1, 512], f32, tag="l", name=f"r{j}_{sub}_{qn}")
                    nc.scalar.activation(rrow[:], lrow[:], AF.Exp, scale=-1.0)
                    r2 = r2pool.tile([64, 512], f32, tag="r2", name=f"r2{j}_{sub}_{qn}")
                    nc.gpsimd.partition_broadcast(r2[:], rrow[:])
                    nc.vector.tensor_tensor(
                        cat_sb[lo:hi, j, qn * 512 : (qn + 1) * 512],
                        pso[0:64, :], r2[:], op=ALU.mult,
                    )
                if j + 2 < PAIRS and qn == 1:
                    fill.extend(proj_quanta(j + 2))

        # --- Remaining output projection (second s-half + leftovers) ---
        feed(len(fill))
        for _, q in o_quanta([4, 5, 6, 7]):
            q()

    nc.compile()
    _STATE["nc"] = nc
    return nc


def _shard(q, k, v, mask, Wq, bq, Wk, bk, Wv, bv, Wo, bo):
    """Build the 8 per-core input maps (host-side layout preparation)."""
    scale = 1.0 / np.sqrt(DK)
    in_maps = []
    for c in range(NCORES):
        b = c // 2
        hh = c % 2
        c0 = hh * 512
        wq_s = (Wq[c0 : c0 + 512, :] * scale).T  # [D, 512]
        wk_s = Wk[c0 : c0 + 512, :].T
        wv_s = Wv[c0 : c0 + 512, :].T
        wo_s = Wo[:, c0 : c0 + 512].T  # [512, D]
        mrow = mask[b, 0, 0, :]
        in_maps.append(
            {
                "xq": np.ascontiguousarray(q[b].T).astype(BF16),
                "xk": np.ascontiguousarray(k[b].T).astype(BF16),
                "xv": np.ascontiguousarray(v[b].T).astype(BF16),
                "wq": np.ascontiguousarray(
                    wq_s.reshape(D, PAIRS, P).transpose(1, 0, 2)
                ).astype(BF16),
                "wk": np.ascontiguousarray(
                    wk_s.reshape(D, PAIRS, P).transpose(1, 0, 2)
                ).astype(BF16),
                "wv": np.ascontiguousarray(wv_s).astype(BF16),
                "wo": np.ascontiguousarray(wo_s).astype(BF16),
                "bq": np.ascontiguousarray(
                    (bq[c0 : c0 + 512] * scale).reshape(PAIRS, P).T, dtype=np.float32
                ),
                "bk": np.ascontiguousarray(
                    bk[c0 : c0 + 512].reshape(PAIRS, P).T, dtype=np.float32
                ),
                "mb": np.ascontiguousarray(
                    np.where(mrow == 0, np.float32(-1e9), np.float32(0.0))
                    .astype(np.float32)
                    .reshape(KT, P)
                    .T
                ),
            }
        )
    return in_maps


def _gather(results, Wv, bv, Wo, bo):
    """Sum per-core partials into the full [B, S, D] output."""
    # Channel-bias correction folded out of the device kernel: the V bias
    # passes through softmax-weighted sums with total weight 1, so its
    # contribution to y is the constant row Wo @ bv.
    corr = (Wo.astype(np.float64) @ bv.astype(np.float64)).astype(np.float32)
    y = np.empty((B, S, D), dtype=np.float32)
    for b in range(B):
        y[b] = results[2 * b]["y"] + results[2 * b + 1]["y"] + corr + bo
    return y


def _run(trace=False, **inputs):
    import time

    from concourse.bass_utils import run_bass_kernel_spmd

    nc = _build()
    args = {k: np.asarray(v) for k, v in inputs.items()}
    in_maps = _shard(**args)
    last_err = None
    for attempt in range(3):
        try:
            res = run_bass_kernel_spmd(
                nc, in_maps, core_ids=list(range(NCORES)), trace=trace
            )
            break
        except Exception as e:  # device occasionally wedges; retry recovers
            last_err = e
            time.sleep(10 * (attempt + 1))
    else:
        raise last_err
    y = _gather(res.results, args["Wv"], args["bv"], args["Wo"], args["bo"])
    return y, res


def kernel(**inputs):
    y, _ = _run(trace=False, **inputs)
    return y


# revision 16
# speedup vs baseline: 1.0278x; 1.0278x over previous
"""Multi-head attention (B=4, S=1024, D=1024, H=16) on 8 TRN2 NeuronCores.

Sharding: batch (4) x head-half (2) -> 8 cores, zero cross-core traffic.
Core c handles batch b = c // 2 and heads [hh*8, hh*8+8) where hh = c % 2.
Each core computes a partial output y_part[s, e] (its 512 channels fed
through its slice of Wo); the host sums the two partials per batch and
adds the bias terms.

On-device pipeline per core (matmul operands bf16, accumulation fp32):
  QT_j = Wq_j' @ xq    [128 dout, 1024 s]   (weights pre-scaled 1/sqrt(dk))
  KT_j = Wk_j' @ xk    [128 dout, 1024 s]
  V'   = xv @ Wv'      [s, 512] natural layout + per-head ones column
  per (pair j, q-chunk qn of 512), per k-tile (8x128), heads row-packed:
    ST   = KhT.T @ QhT            [k 128, q 512]   (K=64, tile_position;
                                   the two heads' streams run concurrently)
    E    = exp(ST + mask_bias)    (ACT, fused mask)
    psO += Vaug.T @ E             [65, q 512]  rows 0-63 = out_h^T, row 64 = denom
    concatT = psO[0:64] * recip(psO[64])   (DVE reciprocal_approx_fast +
                                   gpsimd broadcast + DVE multiply)
  y = concatT.T @ Wo'  [1024 s, 1024 e]

Scheduling: the ACT exp stream (64 x ~1.1us) paces the attention inner
loop, so the PE issue stream weaves "filler" matmul quanta (V projection,
next pair's Q/K projections, first half of the output projection) between
the score matmuls to keep the PE busy during pssp/exp waits. The Scalar
queue carries no DMAs after startup so it runs exps back-to-back.
"""

import os
import sys

sys.path.insert(0, "/opt/trn_rl_repo")

import numpy as np
import ml_dtypes

BF16 = ml_dtypes.bfloat16

B, S, D = 4, 1024, 1024
HEADS = 16
DK = 64
P = 128
NCORES = 8
DCH = D // P       # 8 contraction chunks
PAIRS = 4          # head-pairs per core (8 heads / 2)
QN = 2             # q 512-chunks
KT = 8             # k tiles of 128
VW = 65            # V channels per head + ones column

_STATE = {}


def _build():
    """Build + compile the per-core Bass program (cached)."""
    if "nc" in _STATE:
        return _STATE["nc"]

    import concourse.bass as bass  # noqa: F401
    import concourse.mybir as mybir
    from concourse import bacc
    from concourse import tile

    f32 = mybir.dt.float32
    bf16 = mybir.dt.bfloat16
    AF = mybir.ActivationFunctionType
    ALU = mybir.AluOpType

    # Pin Exp/Ln to the one activation table containing both, so the
    # table-load pass never alternates tables between the softmax exp and
    # the ln/exp reciprocal (each ACT_TABLE_LOAD costs ~1.3us).
    _orig_tables = bacc.get_activation_tables

    def _pinned_tables(arch):
        t = dict(_orig_tables(arch))
        target = "natural_log_exp_and_others"
        if target in t:
            for k in t:
                if k != target:
                    t[k] = t[k] - {AF.Exp, AF.Ln}
        return t

    bacc.get_activation_tables = _pinned_tables

    nc = bacc.Bacc("TRN2", target_bir_lowering=False, debug=False)

    xq_d = nc.dram_tensor("xq", [D, S], bf16, kind="ExternalInput")
    xk_d = nc.dram_tensor("xk", [D, S], bf16, kind="ExternalInput")
    xv_d = nc.dram_tensor("xv", [D, S], bf16, kind="ExternalInput")
    wq_d = nc.dram_tensor("wq", [PAIRS, D, P], bf16, kind="ExternalInput")
    wk_d = nc.dram_tensor("wk", [PAIRS, D, P], bf16, kind="ExternalInput")
    wv_d = nc.dram_tensor("wv", [D, 512], bf16, kind="ExternalInput")
    wo_d = nc.dram_tensor("wo", [512, D], bf16, kind="ExternalInput")
    bq_d = nc.dram_tensor("bq", [P, PAIRS], f32, kind="ExternalInput")
    bk_d = nc.dram_tensor("bk", [P, PAIRS], f32, kind="ExternalInput")
    mb_d = nc.dram_tensor("mb", [P, KT], f32, kind="ExternalInput")
    y_d = nc.dram_tensor("y", [S, D], f32, kind="ExternalOutput")

    from contextlib import ExitStack

    with tile.TileContext(nc) as tc, ExitStack() as ctx:
        const = ctx.enter_context(tc.tile_pool(name="const", bufs=1))
        # Resident tensors
        wv_sb = const.tile([P, DCH, 512], bf16)
        wo_sb = const.tile([P, PAIRS, D], bf16)
        xq_sb = const.tile([P, DCH, S], bf16)
        xk_sb = const.tile([P, DCH, S], bf16)
        xv_sb = const.tile([P, DCH, S], bf16)
        v_sb = const.tile([P, KT, 8 * VW], bf16)
        cat_sb = const.tile([P, PAIRS, S], bf16)
        bq_sb = const.tile([P, PAIRS], f32)
        bk_sb = const.tile([P, PAIRS], f32)
        mb_sb = const.tile([P, KT], f32)

        # Pools
        wqp = ctx.enter_context(tc.tile_pool(name="wqp", bufs=2))
        wkp = ctx.enter_context(tc.tile_pool(name="wkp", bufs=2))
        qtp = ctx.enter_context(tc.tile_pool(name="qtp", bufs=2))
        ktp = ctx.enter_context(tc.tile_pool(name="ktp", bufs=2))
        epool = ctx.enter_context(tc.tile_pool(name="epool", bufs=16))
        rpool = ctx.enter_context(tc.tile_pool(name="rpool", bufs=4))
        r2pool = ctx.enter_context(tc.tile_pool(name="r2pool", bufs=4))
        ypool = ctx.enter_context(tc.tile_pool(name="ypool", bufs=3))
        psacc = ctx.enter_context(tc.tile_pool(name="psacc", bufs=2, space="PSUM"))
        pssp = ctx.enter_context(tc.tile_pool(name="pssp", bufs=2, space="PSUM"))
        psop = ctx.enter_context(tc.tile_pool(name="psop", bufs=2, space="PSUM"))

        # --- Startup DMAs: one big 3D transfer per tensor, spread across
        # queues. Scalar only carries xq (it idles until the first exp).
        xq_r = xq_d.ap().rearrange("(d p) s -> p d s", p=P)
        xk_r = xk_d.ap().rearrange("(d p) s -> p d s", p=P)
        xv_r = xv_d.ap().rearrange("(d p) s -> p d s", p=P)
        wq_r = wq_d.ap().rearrange("j (d p) m -> j p d m", p=P)
        wk_r = wk_d.ap().rearrange("j (d p) m -> j p d m", p=P)

        wq_tiles = [None] * PAIRS
        wk_tiles = [None] * PAIRS

        def load_wqk(j, engine):
            wq_tiles[j] = wqp.tile([P, DCH, P], bf16, tag="wq", name=f"wq{j}")
            engine.dma_start(wq_tiles[j][:], wq_r[j])
            wk_tiles[j] = wkp.tile([P, DCH, P], bf16, tag="wk", name=f"wk{j}")
            engine.dma_start(wk_tiles[j][:], wk_r[j])

        # Chunked DMAs (one per d-slice): a single multi-level-AP DMA runs
        # on one hw queue at ~85GB/s, while 8 chunk DMAs fan out across DMA
        # engines (~4x faster end-to-end). Queue priorities:
        #   scalar (idle until first exp): xq first half
        #   sync:   small consts, xq second half, wq0/wk0, xk tail, wq/wk 1
        #   gpsimd: xk head, xv, wv, wo
        wv_r = wv_d.ap().rearrange("(d p) m -> p d m", p=P)
        nc.sync.dma_start(bq_sb[:], bq_d.ap())
        nc.sync.dma_start(bk_sb[:], bk_d.ap())
        nc.sync.dma_start(mb_sb[:], mb_d.ap())
        load_wqk(0, nc.sync)
        for d in range(4):
            nc.scalar.dma_start(xq_sb[:, d], xq_r[:, d])
            nc.sync.dma_start(xq_sb[:, 4 + d], xq_r[:, 4 + d])
            nc.gpsimd.dma_start(xk_sb[:, d], xk_r[:, d])
        for d in range(4, DCH):
            nc.sync.dma_start(xk_sb[:, d], xk_r[:, d])
        for d in range(DCH):
            nc.gpsimd.dma_start(wv_sb[:, d], wv_r[:, d])
        load_wqk(1, nc.sync)
        for d in range(DCH):
            nc.gpsimd.dma_start(xv_sb[:, d], xv_r[:, d])
        nc.gpsimd.dma_start(wo_sb[:], wo_d.ap().rearrange("(c p) e -> p c e", p=P))

        # Ones columns of V' (denominator trick). memset can't emit a
        # bf16-typed strided set here; stage ones in f32.
        ones_f32 = const.tile([P, KT, 8], f32)
        nc.vector.memset(ones_f32[:], 1.0)
        ones_view = v_sb.rearrange("p t (h c) -> p t h c", c=VW)[:, :, :, 64:65]
        nc.vector.tensor_copy(ones_view, ones_f32[:].unsqueeze(3))

        qt_tiles = [None] * PAIRS
        kt_tiles = [None] * PAIRS

        # ---- Filler machinery: a FIFO of (tag, closure), each emitting
        # ~1-2 matmuls (plus companion DVE/DMA work). feed(n) pops up to n.
        fill = []

        def feed(n):
            for _ in range(min(n, len(fill))):
                fill.pop(0)[1]()

        def drain(tag):
            """Emit every queued quantum up to and including the given tag
            (FIFO order), so later direct emissions see their writers."""
            while any(t == tag for t, _ in fill):
                fill.pop(0)[1]()

        def proj_quanta(j):
            """Q/K projection for pair j as 8 filler quanta."""
            qt = qtp.tile([P, S], bf16, tag="qt", name=f"qt{j}")
            ktt = ktp.tile([P, S], bf16, tag="kt", name=f"kt{j}")
            qt_tiles[j] = qt
            kt_tiles[j] = ktt
            out = []
            for which in ("q", "k"):
                for n in range(QN):
                    wt = wq_tiles[j] if which == "q" else wk_tiles[j]
                    xs = xq_sb if which == "q" else xk_sb
                    dst = qt if which == "q" else ktt
                    bias = bq_sb if which == "q" else bk_sb
                    ps_box = [None]

                    def first(wt=wt, xs=xs, n=n, ps_box=ps_box, j=j, which=which):
                        ps = psacc.tile([P, 512], f32, tag="acc", name=f"ps{which}{j}_{n}")
                        ps_box[0] = ps
                        for d in range(4):
                            nc.tensor.matmul(
                                ps[:], wt[:, d], xs[:, d, n * 512 : (n + 1) * 512],
                                start=(d == 0), stop=False,
                            )

                    def second(wt=wt, xs=xs, n=n, ps_box=ps_box, dst=dst, bias=bias, j=j):
                        ps = ps_box[0]
                        for d in range(4, DCH):
                            nc.tensor.matmul(
                                ps[:], wt[:, d], xs[:, d, n * 512 : (n + 1) * 512],
                                start=False, stop=(d == DCH - 1),
                            )
                        nc.vector.tensor_scalar_add(
                            dst[:, n * 512 : (n + 1) * 512], ps[:], bias[:, j : j + 1]
                        )

                    out.append((f"proj{j}", first))
                    out.append((f"proj{j}", second))
            return out

        def v_quanta():
            """V' projection as 16 filler quanta (2 per s-tile)."""
            out = []
            for st in range(KT):
                ps_box = [None]

                def first(st=st, ps_box=ps_box):
                    ps = psacc.tile([P, 512], f32, tag="acc", name=f"psv{st}")
                    ps_box[0] = ps
                    for d in range(4):
                        nc.tensor.matmul(
                            ps[:], xv_sb[:, d, st * P : (st + 1) * P], wv_sb[:, d],
                            start=(d == 0), stop=False,
                        )

                def second(st=st, ps_box=ps_box):
                    ps = ps_box[0]
                    for d in range(4, DCH):
                        nc.tensor.matmul(
                            ps[:], xv_sb[:, d, st * P : (st + 1) * P], wv_sb[:, d],
                            start=False, stop=(d == DCH - 1),
                        )
                    vview = v_sb[:, st].rearrange("p (h c) -> p h c", c=VW)
                    nc.vector.tensor_copy(
                        vview[:, :, 0:64], ps[:].rearrange("p (h c) -> p h c", c=64)
                    )

                out.append(("v", first))
                out.append(("v", second))
            return out

        y_r = y_d.ap().rearrange("(st p) e -> st p e", p=P)

        def o_quanta(st_list):
            """Output projection for the given s-tiles (1 quantum per (st, en))."""
            out = []
            for st in st_list:
                for en in range(2):
                    def q(st=st, en=en):
                        psy = psacc.tile([P, 512], f32, tag="acc", name=f"psy{st}_{en}")
                        for cc in range(PAIRS):
                            nc.tensor.matmul(
                                psy[:],
                                cat_sb[:, cc, st * P : (st + 1) * P],
                                wo_sb[:, cc, en * 512 : (en + 1) * 512],
                                start=(cc == 0), stop=(cc == PAIRS - 1),
                            )
                        ysb = ypool.tile([P, 512], f32, tag="y", name=f"y{st}_{en}")
                        nc.vector.tensor_copy(ysb[:], psy[:])
                        nc.sync.dma_start(y_r[st][:, en * 512 : (en + 1) * 512], ysb[:])

                    out.append(("o", q))
            return out

        # --- Pair-0 Q/K projections run directly (nothing to overlap yet).
        for _, q in proj_quanta(0):
            q()

        # Filler for pair 0/1 windows: V projection, then pair-1 projections.
        fill += v_quanta()
        fill += proj_quanta(1)

        # --- Head-pair attention loop with weaving ---
        for j in range(PAIRS):
            if j + 2 < PAIRS:
                load_wqk(j + 2, nc.sync)
            drain(f"proj{j}")  # ensure this pair's qt/kt writers are emitted
            qt = qt_tiles[j]
            ktt = kt_tiles[j]
            for qn in range(QN):
                if j == PAIRS - 1 and qn == 1:
                    # First s-half of the output projection only needs qn0
                    # cats (all pairs) -> weave it into the last window.
                    fill.extend(o_quanta([0, 1, 2, 3]))
                ets = []
                for kt in range(KT):
                    pss = pssp.tile([P, 2, 512], f32, tag="s", name=f"pss{j}_{qn}_{kt}")
                    for sub in range(2):
                        lo, hi = sub * 64, (sub + 1) * 64
                        nc.tensor.matmul(
                            pss[:, sub],
                            ktt[lo:hi, kt * P : (kt + 1) * P],
                            qt[lo:hi, qn * 512 : (qn + 1) * 512],
                            start=True, stop=True,
                        )
                    et = epool.tile([P, 2, 512], bf16, tag="e", name=f"e{j}_{qn}_{kt}")
                    nc.scalar.activation(
                        et[:], pss[:], AF.Exp, bias=mb_sb[:, kt : kt + 1], scale=1.0
                    )
                    ets.append(et)
                    feed(3)
                for sub in range(2):
                    h = j * 2 + sub
                    feed(4)
                    pso = psop.tile([VW, 512], f32, tag="o", name=f"pso{j}_{sub}_{qn}")
                    for kt in range(KT):
                        nc.tensor.matmul(
                            pso[:],
                            v_sb[:, kt, h * VW : (h + 1) * VW],
                            ets[kt][:, sub],
                            start=(kt == 0), stop=(kt == KT - 1),
                        )
                    lo, hi = sub * 64, (sub + 1) * 64
                    # Normalize: 1/denom as exp(-ln(denom)) on ACT, gpsimd
                    # partition-broadcast, then the multiply evicts psO.
                    lrow = rpool.tile([1, 512], f32, tag="l", name=f"l{j}_{sub}_{qn}")
                    nc.scalar.activation(lrow[:], pso[64:65, :], AF.Ln)
                    rrow = rpool.tile([1, 512], f32, tag="l", name=f"r{j}_{sub}_{qn}")
                    nc.scalar.activation(rrow[:], lrow[:], AF.Exp, scale=-1.0)
                    r2 = r2pool.tile([64, 512], f32, tag="r2", name=f"r2{j}_{sub}_{qn}")
                    nc.gpsimd.partition_broadcast(r2[:], rrow[:])
                    nc.vector.tensor_tensor(
                        cat_sb[lo:hi, j, qn * 512 : (qn + 1) * 512],
                        pso[0:64, :], r2[:], op=ALU.mult,
                    )
                if j + 2 < PAIRS and qn == 1:
                    fill.extend(proj_quanta(j + 2))

        # --- Remaining output projection (second s-half + leftovers) ---
        feed(len(fill))
        for _, q in o_quanta([4, 5, 6, 7]):
            q()

    nc.compile()
    _STATE["nc"] = nc
    return nc


def _shard(q, k, v, mask, Wq, bq, Wk, bk, Wv, bv, Wo, bo):
    """Build the 8 per-core input maps (host-side layout preparation)."""
    scale = 1.0 / np.sqrt(DK)
    in_maps = []
    for c in range(NCORES):
        b = c // 2
        hh = c % 2
        c0 = hh * 512
        wq_s = (Wq[c0 : c0 + 512, :] * scale).T  # [D, 512]
        wk_s = Wk[c0 : c0 + 512, :].T
        wv_s = Wv[c0 : c0 + 512, :].T
        wo_s = Wo[:, c0 : c0 + 512].T  # [512, D]
        mrow = mask[b, 0, 0, :]
        in_maps.append(
            {
                "xq": np.ascontiguousarray(q[b].T).astype(BF16),
                "xk": np.ascontiguousarray(k[b].T).astype(BF16),
                "xv": np.ascontiguousarray(v[b].T).astype(BF16),
                "wq": np.ascontiguousarray(
                    wq_s.reshape(D, PAIRS, P).transpose(1, 0, 2)
                ).astype(BF16),
                "wk": np.ascontiguousarray(
                    wk_s.reshape(D, PAIRS, P).transpose(1, 0, 2)
                ).astype(BF16),
                "wv": np.ascontiguousarray(wv_s).astype(BF16),
                "wo": np.ascontiguousarray(wo_s).astype(BF16),
                "bq": np.ascontiguousarray(
                    (bq[c0 : c0 + 512] * scale).reshape(PAIRS, P).T, dtype=np.float32
                ),
                "bk": np.ascontiguousarray(
                    bk[c0 : c0 + 512].reshape(PAIRS, P).T, dtype=np.float32
                ),
                "mb": np.ascontiguousarray(
                    np.where(mrow == 0, np.float32(-1e9), np.float32(0.0))
                    .astype(np.float32)
                    .reshape(KT, P)
                    .T
                ),
            }
        )
    return in_maps


def _gather(results, Wv, bv, Wo, bo):
    """Sum per-core partials into the full [B, S, D] output."""
    # Channel-bias correction folded out of the device kernel: the V bias
    # passes through softmax-weighted sums with total weight 1, so its
    # contribution to y is the constant row Wo @ bv.
    corr = (Wo.astype(np.float64) @ bv.astype(np.float64)).astype(np.float32)
    y = np.empty((B, S, D), dtype=np.float32)
    for b in range(B):
        y[b] = results[2 * b]["y"] + results[2 * b + 1]["y"] + corr + bo
    return y


def _run(trace=False, **inputs):
    import time

    from concourse.bass_utils import run_bass_kernel_spmd

    nc = _build()
    args = {k: np.asarray(v) for k, v in inputs.items()}
    in_maps = _shard(**args)
    last_err = None
    for attempt in range(3):
        try:
            res = run_bass_kernel_spmd(
                nc, in_maps, core_ids=list(range(NCORES)), trace=trace
            )
            break
        except Exception as e:  # device occasionally wedges; retry recovers
            last_err = e
            time.sleep(10 * (attempt + 1))
    else:
        raise last_err
    y = _gather(res.results, args["Wv"], args["bv"], args["Wo"], args["bo"])
    return y, res


def kernel(**inputs):
    y, _ = _run(trace=False, **inputs)
    return y
